# revision 1
# baseline (speedup 1.0000x reference)
"""Longformer decoder (4 layers, sliding-window causal attention) on 8 trn2 cores.

Sharding: 4096 tokens (B=2 x S=2048) split into 8 contiguous chunks of 512
(core = b*4 + chunk). Activations are kept d-major ([dim, token], dim on
partitions) so every matmul is weights-stationary with no transposes.
Attention needs a 256-token left halo of K/V per layer: layer 0 computes it
locally from the embedding gather; layers 1-3 AllGather the residual-stream
halo over 4-core groups. Final projection is vocab-sharded: after an 8-core
AllGather of the final LN output, each core computes all 4096 tokens x its
4000-vocab slice of w_out (padded to 4096). Matmuls run in float32r
(full-speed fp32 mode, ~1.5e-4 rel err).
"""
import os
import sys

import numpy as np

for _p in ("/opt/trn_rl_repo", "/root/.axon_site/_ro/trn_rl_repo"):
    if os.path.isdir(_p) and _p not in sys.path:
        sys.path.insert(0, _p)

import concourse.bass as bass
import concourse.mybir as mybir
import concourse.tile as tile
from concourse import bacc
from concourse.bass import ts, ds
from concourse.bass_utils import run_bass_kernel_spmd
from concourse.masks import make_identity

F32 = mybir.dt.float32
F32R = mybir.dt.float32r
F16 = mybir.dt.float16
I32 = mybir.dt.int32
MDT = F16 if os.environ.get("KMMDT", "f16") == "f16" else F32R
AF = mybir.ActivationFunctionType
OP = mybir.AluOpType

B, S, V, D, H, NL, MLPD = 2, 2048, 32000, 512, 8, 4, 2048
DH = D // H            # 64
HALF = 256             # attention half-window (WIN // 2)
P = 128
NCORES = 8
CHUNK = 512            # own tokens per core
W = CHUNK + HALF       # 768 = halo + own
DT = D // P            # 4 d-tiles
MT = MLPD // P         # 16 mlp tiles
VSH = 4096             # padded vocab shard (actual 4000)
NTOK = B * S           # 4096
VV = V // NCORES       # 4000 valid vocab per core
GROUPS = [[0, 1, 2, 3], [4, 5, 6, 7]]
EXP_SHIFT = 2.0
SCALE = float(1.0 / np.sqrt(DH))

_CACHE = {}


# ================================================================ builder
def _build():
    nc = bacc.Bacc("TRN2", target_bir_lowering=False, debug=False,
                   num_devices=NCORES)

    ein = lambda n, sh, dt=F32: nc.dram_tensor(n, sh, dt, kind="ExternalInput")
    io = dict(
        wq=ein("wq", [NL, D, D], MDT), wk=ein("wk", [NL, D, D], MDT),
        wv=ein("wv", [NL, D, D], MDT), wo=ein("wo", [NL, D, D], MDT),
        w1=ein("w1", [NL, D, MLPD], MDT), w2=ein("w2", [NL, MLPD, D], MDT),
        b1=ein("b1", [NL, MLPD]), b2=ein("b2", [NL, D]),
        ln1_s=ein("ln1_s", [NL, D]), ln1_b=ein("ln1_b", [NL, D]),
        ln2_s=ein("ln2_s", [NL, D]), ln2_b=ein("ln2_b", [NL, D]),
        lnf_s=ein("lnf_s", [1, D]), lnf_b=ein("lnf_b", [1, D]),
        w_out_sl=ein("w_out_sl", [D, VSH], MDT), b_out_sl=ein("b_out_sl", [1, VSH]),
        embed=ein("embed", [V, D]),
        idx_in=ein("idx_in", [P, W // P], I32),
        pe_dm=ein("pe_dm", [D, W]),
        masks=ein("masks", [2, 4, P, 256]),
        halo_offs=ein("halo_offs", [P, DT], I32),
        out=nc.dram_tensor("logits_vm", [VSH, NTOK], F32, kind="ExternalOutput"),
    )
    if os.environ.get("KDEBUG") == "1":
        io["xdump"] = nc.dram_tensor("xdump", [NL + 1, D, CHUNK], F32, kind="ExternalOutput")
        io["ydump"] = nc.dram_tensor("ydump", [D, W], F32, kind="ExternalOutput")
        io["adump"] = nc.dram_tensor("adump", [D, CHUNK], F32, kind="ExternalOutput")

    with tile.TileContext(nc) as tc, nc.allow_low_precision(reason="f32r rounding"):
        _emit(nc, tc, io)
    nc.compile()
    return nc


def _emit(nc, tc, io):
    cpool = tc.alloc_tile_pool(name="const", bufs=1)
    xpool = tc.alloc_tile_pool(name="xres", bufs=1)
    ps_a = tc.alloc_tile_pool(name="ps_a", bufs=2, space="PSUM")
    ps_b = tc.alloc_tile_pool(name="ps_b", bufs=4, space="PSUM")
    ps_c = tc.alloc_tile_pool(name="ps_c", bufs=2, space="PSUM")
    drp = tc.alloc_tile_pool(name="drbounce", bufs=1, space="DRAM")

    # ------------------------------------------------ constants
    ones_f = cpool.tile([P, P], F32, tag="ones_f")
    nc.vector.memset(ones_f[:], 1.0)
    ones = cpool.tile([P, P], MDT, tag="ones")
    nc.vector.tensor_copy(out=ones[:], in_=ones_f[:])
    ones_r = cpool.tile([P, P], F32R, tag="ones_r")
    nc.vector.tensor_copy(out=ones_r[:], in_=ones_f[:])
    negb = cpool.tile([P, 1], F32, tag="negb")
    nc.vector.memset(negb[:], EXP_SHIFT)
    epsb = cpool.tile([P, 1], F32, tag="epsb")
    nc.vector.memset(epsb[:], 1e-6)
    l1s = cpool.tile([P, NL, DT], F32, tag="l1s")
    l1b = cpool.tile([P, NL, DT], F32, tag="l1b")
    l2s = cpool.tile([P, NL, DT], F32, tag="l2s")
    l2b = cpool.tile([P, NL, DT], F32, tag="l2b")
    lfs = cpool.tile([P, DT], F32, tag="lfs")
    lfb = cpool.tile([P, DT], F32, tag="lfb")
    nc.sync.dma_start(out=l1s[:], in_=io["ln1_s"].ap().rearrange("l (t p) -> p l t", p=P))
    nc.sync.dma_start(out=l1b[:], in_=io["ln1_b"].ap().rearrange("l (t p) -> p l t", p=P))
    nc.sync.dma_start(out=l2s[:], in_=io["ln2_s"].ap().rearrange("l (t p) -> p l t", p=P))
    nc.sync.dma_start(out=l2b[:], in_=io["ln2_b"].ap().rearrange("l (t p) -> p l t", p=P))
    nc.sync.dma_start(out=lfs[:], in_=io["lnf_s"].ap().rearrange("o (t p) -> p (o t)", p=P))
    nc.sync.dma_start(out=lfb[:], in_=io["lnf_b"].ap().rearrange("o (t p) -> p (o t)", p=P))
    b1t = cpool.tile([P, NL, MT], F32, tag="b1t")
    b2t = cpool.tile([P, NL, DT], F32, tag="b2t")
    bot = cpool.tile([P, VSH // P], F32, tag="bot")
    nc.sync.dma_start(out=b1t[:], in_=io["b1"].ap().rearrange("l (m p) -> p l m", p=P))
    nc.sync.dma_start(out=b2t[:], in_=io["b2"].ap().rearrange("l (t p) -> p l t", p=P))
    nc.sync.dma_start(out=bot[:], in_=io["b_out_sl"].ap().rearrange("o (t p) -> p (o t)", p=P))
    masks = cpool.tile([P, 2, 4, 256], F32, tag="masks")
    nc.sync.dma_start(out=masks[:], in_=io["masks"].ap().rearrange("a b p q -> p a b q"))
    hoffs = cpool.tile([P, DT], I32, tag="hoffs")
    nc.sync.dma_start(out=hoffs[:], in_=io["halo_offs"].ap())

    # residual stream (own 512 tokens, d-major) + per-layer halo
    x = xpool.tile([P, DT, CHUNK], F32, tag="x")
    xh = xpool.tile([P, DT, HALF], F32, tag="xh")

    # ------------------------------------------------ embedding
    with tc.tile_pool(name="embed", bufs=1) as epool:
        ident = epool.tile([P, P], F32, tag="ident")
        make_identity(nc, ident[:])
        pe = epool.tile([P, DT, W], F32, tag="pe")
        nc.sync.dma_start(out=pe[:], in_=io["pe_dm"].ap().rearrange("(t p) m -> p t m", p=P))
        idxt = epool.tile([P, W // P], I32, tag="idxt")
        nc.sync.dma_start(out=idxt[:], in_=io["idx_in"].ap())
        with tc.tile_pool(name="gath", bufs=2) as gpool:
            for g in range(W // P):
                gt = gpool.tile([P, D], F32, tag="gt")
                nc.gpsimd.indirect_dma_start(
                    out=gt[:], out_offset=None, in_=io["embed"].ap(),
                    in_offset=bass.IndirectOffsetOnAxis(ap=idxt[:, g:g + 1], axis=0),
                )
                for dt in range(DT):
                    pt = ps_a.tile([P, P], F32, tag="ps_a")
                    nc.tensor.transpose(pt[:], gt[:, ts(dt, P)], ident[:])
                    dst = xh[:, dt, ts(g, P)] if g < 2 else x[:, dt, ts(g - 2, P)]
                    nc.vector.tensor_add(out=dst, in0=pt[:], in1=pe[:, dt, ts(g, P)])

    def dump_x(slot):
        if "xdump" in io:
            nc.sync.dma_start(out=io["xdump"].ap()[slot].rearrange("(t p) m -> p t m", p=P),
                              in_=x[:, :, :])
    dump_x(0)

    # ------------------------------------------------ layer pools
    lp = tc.alloc_tile_pool(name="layers", bufs=1)
    tp = tc.alloc_tile_pool(name="ltrans", bufs=2)
    lp3 = tc.alloc_tile_pool(name="ltrans3", bufs=3)

    def emit_ln(srcs, y, s_of, b_of):
        """LN over d. srcs: list of (fn(dt)->AP[128,width], y_col0, width).
        y: [P, DT, w_tok] F32R out. s_of/b_of: fn(dt)->AP[P,1]."""
        srcs2 = []
        for fn, col0, width in srcs:
            for o in range(0, width, 256):
                srcs2.append((lambda dt, fn=fn, o=o: fn(dt)[:, ds(o, 256)], col0 + o, 256))
        for fn, col0, width in srcs2:
            sx = ps_a.tile([1, 512], F32, tag="ps_a")
            sxx = ps_a.tile([1, 512], F32, tag="ps_a")
            for dt in range(DT):
                xr = lp3.tile([P, 512], MDT, tag="ln_xr", bufs=2)
                xsq = lp3.tile([P, 512], MDT, tag="ln_xsq", bufs=2)
                nc.gpsimd.tensor_copy(out=xr[:, :width], in_=fn(dt))
                nc.vector.tensor_mul(out=xsq[:, :width], in0=fn(dt), in1=fn(dt))
                nc.tensor.matmul(out=sx[:, :width], lhsT=ones[:, 0:1], rhs=xr[:, :width],
                                 start=(dt == 0), stop=(dt == DT - 1))
                nc.tensor.matmul(out=sxx[:, :width], lhsT=ones[:, 0:1], rhs=xsq[:, :width],
                                 start=(dt == 0), stop=(dt == DT - 1))
            mu = lp3.tile([1, 512], F32R, tag="ln_mu", bufs=2)
            mu2 = lp3.tile([1, 512], F32, tag="ln_mu2", bufs=1)
            var = lp3.tile([1, 512], F32, tag="ln_var", bufs=1)
            sd = lp3.tile([1, 512], F32, tag="ln_sd", bufs=1)
            rstd = lp3.tile([1, 512], F32R, tag="ln_rstd", bufs=2)
            nc.vector.tensor_scalar_mul(out=mu[:, :width], in0=sx[:, :width], scalar1=1.0 / D)
            nc.vector.tensor_mul(out=mu2[:, :width], in0=mu[:, :width], in1=mu[:, :width])
            # var = sxx/D - mu^2
            nc.vector.scalar_tensor_tensor(
                out=var[:, :width], in0=sxx[:, :width], scalar=1.0 / D,
                in1=mu2[:, :width], op0=OP.mult, op1=OP.subtract)
            nc.scalar.activation(sd[:, :width], var[:, :width], AF.Sqrt, bias=epsb[0:1, :], scale=1.0)
            nc.vector.reciprocal(out=rstd[:, :width], in_=sd[:, :width])
            pmu = ps_a.tile([P, 512], F32, tag="ps_a")
            nc.tensor.matmul(out=pmu[:, :width], lhsT=ones_r[0:1, :], rhs=mu[:, :width],
                             start=True, stop=True)
            prs = ps_a.tile([P, 512], F32, tag="ps_a")
            nc.tensor.matmul(out=prs[:, :width], lhsT=ones_r[0:1, :], rhs=rstd[:, :width],
                             start=True, stop=True)
            mu_b = lp3.tile([P, 512], F32, tag="ln_mub", bufs=1)
            rs_b = lp3.tile([P, 512], F32, tag="ln_rsb", bufs=1)
            nc.vector.tensor_copy(out=mu_b[:, :width], in_=pmu[:, :width])
            nc.vector.tensor_copy(out=rs_b[:, :width], in_=prs[:, :width])
            for dt in range(DT):
                scr = lp3.tile([P, 512], F32, tag="ln_scr", bufs=2)
                nc.vector.tensor_sub(out=scr[:, :width], in0=fn(dt), in1=mu_b[:, :width])
                nc.vector.tensor_mul(out=scr[:, :width], in0=scr[:, :width], in1=rs_b[:, :width])
                nc.vector.tensor_scalar(out=y[:, dt, ds(col0, width)], in0=scr[:, :width],
                                        scalar1=s_of(dt), scalar2=b_of(dt),
                                        op0=OP.mult, op1=OP.add)

    def load_w(dram_ap, tag_r, shape3, rpool=None):
        wr = (rpool or tp).tile(shape3, MDT, tag=tag_r)
        nc.sync.dma_start(out=wr[:], in_=dram_ap)
        return wr

    # ------------------------------------------------ transformer layers
    _knl = int(os.environ.get("KNL", NL))
    _skipatt = os.environ.get("KSKIPATT") == "1"
    _skipmlp = os.environ.get("KSKIPMLP") == "1"
    _skipfin = os.environ.get("KSKIPFIN") == "1"
    for l in range(_knl):
        y = lp.tile([P, DT, W], MDT, tag="y")
        emit_ln(
            srcs=[(lambda dt: x[:, dt, :], HALF, CHUNK),
                  (lambda dt: xh[:, dt, :], 0, HALF)],
            y=y, s_of=lambda dt: l1s[:, l % NL, dt:dt + 1], b_of=lambda dt: l1b[:, l % NL, dt:dt + 1])

        # --- projections (weights stationary, d-major out)
        wq_r = load_w(io["wq"].ap()[l % NL].rearrange("(t p) m -> p t m", p=P), "wr", [P, DT, D])
        qr = lp.tile([P, DT, CHUNK], MDT, tag="qr")
        for do in range(DT):
            pq = ps_a.tile([P, CHUNK], F32, tag="ps_a")
            for dt in range(DT):
                nc.tensor.matmul(out=pq[:], lhsT=wq_r[:, dt, ts(do, P)],
                                 rhs=y[:, dt, ds(HALF, CHUNK)],
                                 start=(dt == 0), stop=(dt == DT - 1))
            nc.vector.tensor_copy(out=qr[:, do, :], in_=pq[:])

        wk_r = load_w(io["wk"].ap()[l % NL].rearrange("(t p) m -> p t m", p=P), "wr", [P, DT, D])
        kr = lp.tile([P, DT, W], MDT, tag="kr")
        for do in range(DT):
            for c0, cw in ((HALF, CHUNK), (0, HALF)):
                pk = ps_a.tile([P, CHUNK], F32, tag="ps_a")
                for dt in range(DT):
                    nc.tensor.matmul(out=pk[:, :cw], lhsT=wk_r[:, dt, ts(do, P)],
                                     rhs=y[:, dt, ds(c0, cw)],
                                     start=(dt == 0), stop=(dt == DT - 1))
                nc.vector.tensor_copy(out=kr[:, do, ds(c0, cw)], in_=pk[:, :cw])

        wv_r = load_w(io["wv"].ap()[l % NL].rearrange("(t p) m -> p t m", p=P), "wr", [P, DT, D])
        vt = [lp.tile([P, H * (DH + 1)], MDT, tag=f"vt{t}", name=f"vt{t}") for t in range(W // P)]
        for t in range(W // P):
            pv = ps_a.tile([P, D], F32, tag="ps_a")
            for dt in range(DT):
                nc.tensor.matmul(out=pv[:], lhsT=y[:, dt, ts(t, P)], rhs=wv_r[:, dt, :],
                                 start=(dt == 0), stop=(dt == DT - 1))
            vtv = vt[t][:].rearrange("p (h c) -> p h c", c=DH + 1)
            nc.vector.tensor_copy(out=vtv[:, :, 0:DH],
                                  in_=pv[:].rearrange("p (h c) -> p h c", c=DH))
            nc.vector.tensor_copy(out=vtv[:, :, DH:DH + 1], in_=ones[:, 0:H])

        # --- sliding-window attention
        attr = lp.tile([P, DT, CHUNK], MDT, tag="attr")
        for h in (range(0) if _skipatt else range(H)):
            r0 = (h % 2) * DH
            dto = h // 2
            for qB in range(2):
                pa = ps_c.tile([DH + 1, 256], F32, tag="ps_c")
                for j in range(4):
                    kt = qB * 2 + j
                    pscore = ps_b.tile([P, 256], F32, tag="ps_b")
                    nc.tensor.matmul(
                        out=pscore[:],
                        lhsT=kr[ds(r0, DH), dto, ds(qB * 256 + j * P, P)],
                        rhs=qr[ds(r0, DH), dto, ds(qB * 256, 256)],
                        start=True, stop=True)
                    ej = lp3.tile([P, 256], MDT, tag="ej", bufs=4)
                    nc.scalar.activation(ej[:], pscore[:], AF.Exp, bias=negb[:], scale=SCALE)
                    nc.vector.tensor_mul(out=ej[:], in0=ej[:], in1=masks[:, qB, j, :])
                    nc.tensor.matmul(out=pa[:], lhsT=vt[kt][:, ds(h * (DH + 1), DH + 1)],
                                     rhs=ej[:], start=(j == 0), stop=(j == 3))
                rr = lp3.tile([1, 256], F32R, tag="rr")
                nc.vector.reciprocal(out=rr[:], in_=pa[DH:DH + 1, :])
                pbc = ps_c.tile([DH, 256], F32, tag="ps_c")
                nc.tensor.matmul(out=pbc[:], lhsT=ones_r[0:1, 0:DH], rhs=rr[:],
                                 start=True, stop=True)
                bcs = lp3.tile([DH, 256], MDT, tag="bcs")
                nc.vector.tensor_copy(out=bcs[:], in_=pbc[:])
                nc.vector.tensor_mul(out=attr[ds(r0, DH), dto, ds(qB * 256, 256)],
                                     in0=pa[0:DH, :], in1=bcs[:])
        if _skipatt:
            for dt in range(DT):
                nc.vector.tensor_copy(out=attr[:, dt, :], in_=qr[:, dt, :])

        if l == 0 and "ydump" in io:
            yd = lp3.tile([P, DT, W], F32, tag="ydump_t", bufs=1)
            nc.vector.tensor_copy(out=yd[:], in_=y[:])
            nc.sync.dma_start(out=io["ydump"].ap().rearrange("(t p) m -> p t m", p=P), in_=yd[:])
        if l == 0 and "adump" in io:
            ad = lp3.tile([P, DT, CHUNK], F32, tag="adump_t", bufs=1)
            nc.vector.tensor_copy(out=ad[:], in_=attr[:])
            nc.sync.dma_start(out=io["adump"].ap().rearrange("(t p) m -> p t m", p=P), in_=ad[:])

        # --- output projection + residual
        wo_r = load_w(io["wo"].ap()[l % NL].rearrange("(t p) m -> p t m", p=P), "wr", [P, DT, D])
        for do in range(DT):
            po = ps_a.tile([P, CHUNK], F32, tag="ps_a")
            for dt in range(DT):
                nc.tensor.matmul(out=po[:], lhsT=wo_r[:, dt, ts(do, P)],
                                 rhs=attr[:, dt, :], start=(dt == 0), stop=(dt == DT - 1))
            nc.vector.tensor_add(out=x[:, do, :], in0=x[:, do, :], in1=po[:])

        # --- LN2 + MLP
        y2 = lp.tile([P, DT, CHUNK], MDT, tag="y2")
        emit_ln(srcs=[(lambda dt: x[:, dt, :], 0, CHUNK)], y=y2,
                s_of=lambda dt: l2s[:, l % NL, dt:dt + 1], b_of=lambda dt: l2b[:, l % NL, dt:dt + 1])

        pb = [ps_b.tile([P, CHUNK], F32, tag="ps_b", name=f"pb{i}") for i in range(DT)]
        w1r = lp.tile([P, DT, MLPD], MDT, tag="w1r")
        nc.sync.dma_start(out=w1r[:], in_=io["w1"].ap()[l % NL].rearrange("(t p) m -> p t m", p=P))
        w2r = lp.tile([P, MT, D], MDT, tag="w2r")
        nc.sync.dma_start(out=w2r[:], in_=io["w2"].ap()[l % NL].rearrange("(t p) m -> p t m", p=P))

        def emit_mlp2(m, hm):
            for do in range(DT):
                nc.tensor.matmul(out=pb[do][:], lhsT=w2r[:, m, ts(do, P)],
                                 rhs=hm[:], start=(m == 0), stop=(m == MT - 1))

        hist = []
        for m in (range(0) if _skipmlp else range(MT)):
            p1 = ps_a.tile([P, CHUNK], F32, tag="ps_a")
            for dt in range(DT):
                nc.tensor.matmul(out=p1[:], lhsT=w1r[:, dt, ts(m, P)],
                                 rhs=y2[:, dt, :],
                                 start=(dt == 0), stop=(dt == DT - 1))
            hm = lp3.tile([P, CHUNK], MDT, tag="hm", bufs=3)
            nc.scalar.activation(hm[:], p1[:], AF.Gelu_apprx_tanh,
                                 bias=b1t[:, l % NL, m:m + 1], scale=1.0)
            hist.append((m, hm))
            if len(hist) > 2:
                emit_mlp2(*hist.pop(0))
        for mm_, hh_ in hist:
            emit_mlp2(mm_, hh_)
        # residual (+b2), then send halo for next layer
        for do in (range(0) if _skipmlp else range(DT)):
            nc.vector.scalar_tensor_tensor(
                out=x[:, do, :], in0=pb[do][:],
                scalar=b2t[:, l % NL, do:do + 1], in1=x[:, do, :],
                op0=OP.add, op1=OP.add)
        if l < NL - 1:
            agin = drp.tile([D, HALF], F32, tag=f"agin{l}")
            agout = drp.tile([len(GROUPS[0]) * D, HALF], F32, tag=f"agout{l}")
            nc.sync.dma_start(out=agin[:].rearrange("(t p) m -> p t m", p=P),
                              in_=x[:, :, ds(HALF, HALF)])
            nc.gpsimd.collective_compute(
                "AllGather", OP.bypass, replica_groups=GROUPS,
                ins=[agin.opt()], outs=[agout.opt()])
        if l < NL - 1:
            for dt in range(DT):
                nc.gpsimd.indirect_dma_start(
                    out=xh[:, dt, :], out_offset=None, in_=agout[:],
                    in_offset=bass.IndirectOffsetOnAxis(ap=hoffs[:, dt:dt + 1], axis=0))
        dump_x(l + 1)

    # ------------------------------------------------ final LN + allgather
    yf = lp.tile([P, DT, CHUNK], MDT, tag="y")
    emit_ln(srcs=[(lambda dt: x[:, dt, :], 0, CHUNK)], y=yf,
            s_of=lambda dt: lfs[:, dt:dt + 1], b_of=lambda dt: lfb[:, dt:dt + 1])
    yfd = drp.tile([D, CHUNK], MDT, tag="yfd")
    nc.sync.dma_start(out=yfd[:].rearrange("(t p) m -> p t m", p=P), in_=yf[:])
    yfg = drp.tile([NCORES * D, CHUNK], MDT, tag="yfg", addr_space="Shared")
    nc.gpsimd.collective_compute(
        "AllGather", OP.bypass, replica_groups=[list(range(NCORES))],
        ins=[yfd.opt()], outs=[yfg.opt()])

    lp3.release()
    tp.release()
    lp.release()

    # ------------------------------------------------ vocab-sharded logits
    with tc.tile_pool(name="final", bufs=1) as fpool, \
         tc.tile_pool(name="ftrans", bufs=3) as ftp:
        yall = fpool.tile([P, DT, NTOK], MDT, tag="yall")
        for dt in range(DT):
            nc.sync.dma_start(
                out=yall[:, dt, :].rearrange("p (c t) -> p c t", c=NCORES),
                in_=yfg[:].rearrange("(c q p) t -> p q c t", q=DT, p=P)[:, dt, :, :])
        for _frep in range(int(os.environ.get("KFINREP", "1"))):
            for v_i in (range(0) if _skipfin else range(VSH // P)):
                fwr = ftp.tile([P, DT, P], MDT, tag="fwr")
                nc.sync.dma_start(out=fwr[:], in_=io["w_out_sl"].ap()[:, ts(v_i, P)]
                                  .rearrange("(t p) m -> p t m", p=P))
                for tb in range(NTOK // 512):
                    pf = ps_a.tile([P, 512], F32, tag="ps_a")
                    for dt in range(DT):
                        nc.tensor.matmul(out=pf[:], lhsT=fwr[:, dt, :],
                                         rhs=yall[:, dt, ts(tb, 512)],
                                         start=(dt == 0), stop=(dt == DT - 1))
                    ot = ftp.tile([P, 512], F32, tag="fot")
                    if tb % 2 == 0:
                        nc.scalar.activation(ot[:], pf[:], AF.Identity,
                                             bias=bot[:, v_i:v_i + 1], scale=1.0)
                    else:
                        nc.vector.tensor_scalar_add(out=ot[:], in0=pf[:],
                                                    scalar1=bot[:, v_i:v_i + 1])
                    nc.sync.dma_start(out=io["out"].ap()[ts(v_i, P), ts(tb, 512)],
                                      in_=ot[:])

    drp.release()
    ps_c.release()
    ps_b.release()
    ps_a.release()
    xpool.release()
    cpool.release()


# ================================================================ host side
def _pe_table():
    pos = np.arange(S, dtype=np.float32)[:, None]
    div = np.exp(np.arange(0, D, 2, dtype=np.float32) * -(np.log(10000.0) / D))
    pe = np.zeros((S, D), dtype=np.float32)
    pe[:, 0::2] = np.sin(pos * div)
    pe[:, 1::2] = np.cos(pos * div)
    return pe


def _in_maps(inputs):
    inp = np.asarray(inputs["inputs"]).astype(np.int32)
    ids = np.pad(inp, ((0, 0), (1, 0)))[:, :-1].astype(np.int32)
    pe = _pe_table()
    wout = np.asarray(inputs["w_out"], dtype=np.float32).astype(np.float16)
    bout = np.asarray(inputs["b_out"], dtype=np.float32)
    shared = {k: np.ascontiguousarray(np.asarray(inputs[k], dtype=np.float32))
              for k in ("embed", "b1", "b2", "ln1_s", "ln1_b", "ln2_s", "ln2_b")}
    for k in ("wq", "wk", "wv", "wo", "w1", "w2"):
        shared[k] = np.ascontiguousarray(
            np.asarray(inputs[k], dtype=np.float32).astype(np.float16))
    shared["lnf_s"] = np.asarray(inputs["lnf_s"], np.float32).reshape(1, D)
    shared["lnf_b"] = np.asarray(inputs["lnf_b"], np.float32).reshape(1, D)

    maps = []
    qi = np.arange(256)[None, :]
    ki = np.arange(P)[:, None]
    for c in range(NCORES):
        b, ch = divmod(c, NCORES // B)
        t0 = ch * CHUNK
        lo = t0 - HALF
        ids768 = np.zeros(W, np.int32)
        pe768 = np.zeros((W, D), np.float32)
        s0 = max(0, lo)
        ids768[s0 - lo:] = ids[b, s0:t0 + CHUNK]
        pe768[s0 - lo:] = pe[s0:t0 + CHUNK]
        m = np.zeros((2, 4, P, 256), np.float32)
        for qB in range(2):
            for j in range(4):
                w = 256 + qi - (j * P + ki)      # u_q - u_k
                ok = (w >= 0) & (w <= HALF)
                if ch == 0:
                    ok = ok & ((lo + qB * 256 + j * P + ki) >= 0)
                m[qB, j] = ok.astype(np.float32)
        src = ch - 1 if ch > 0 else 0
        hoffs = (src * D + np.arange(DT)[None, :] * P
                 + np.arange(P)[:, None]).astype(np.int32)
        vlo = c * VV
        wsl = np.zeros((D, VSH), np.float16)
        wsl[:, :VV] = wout[:, vlo:vlo + VV]
        bsl = np.zeros((1, VSH), np.float32)
        bsl[0, :VV] = bout[vlo:vlo + VV]
        mp = dict(shared)
        mp.update(
            idx_in=np.ascontiguousarray(ids768.reshape(W // P, P).T),
            pe_dm=np.ascontiguousarray(pe768.T),
            masks=m, halo_offs=hoffs, w_out_sl=wsl, b_out_sl=bsl)
        maps.append(mp)
    return maps


def kernel(**inputs):
    nc = _CACHE.get("nc")
    if nc is None:
        nc = _build()
        _CACHE["nc"] = nc
    maps = _in_maps(inputs)
    res = run_bass_kernel_spmd(nc, maps, list(range(NCORES))).results
    full = np.empty((NTOK, V), np.float32)
    for c in range(NCORES):
        full[:, c * VV:(c + 1) * VV] = res[c]["logits_vm"][:VV, :].T
    return full.reshape(B, S, V)



# revision 19
# speedup vs baseline: 1.2968x; 1.2968x over previous
"""Longformer decoder (4 layers, sliding-window causal attention) on 8 trn2 cores.

Sharding: 4096 tokens (B=2 x S=2048) split into 8 contiguous chunks of 512
(core = b*4 + chunk). Activations are kept d-major ([dim, token], dim on
partitions) so every matmul is weights-stationary with no transposes.

v2 structure (vs v1 baseline):
- LN affine (scale/bias) folded into the projection weights host-side; the
  kernel's LN emits the unaffined z=(x-mu)*rstd, with biases re-applied via
  scalar-engine Identity copies (per-partition bias) where needed.
- LN statistics matmuls run on f32r bitcasts of the residual directly (no
  gpsimd f16 staging copies).
- reciprocal_approx_fast for all softmax/LN reciprocals.
- Attention restructured per-head into kt (key-tile) granularity with f16
  masks; per layer, all heads' own-key work (qB=1) runs first so the x-halo
  AllGather from the previous layer is consumed ~40us into the layer.
- Weights double-buffered and prefetched one layer ahead; w_out cached in
  SBUF during the last layer.
- Final logits loop runs the core's own 512-token block first (directly off
  the local yf) while the 8-way AllGather is in flight; remaining blocks are
  imported with per-core indirect DMAs. Output token-blocks are rotated
  per-core ((c+j)%8) so the SPMD program stays identical; host unrotates.
- Logits written f16 (tolerance 2e-2; f16 quantization ~5e-4).
"""
import os
import sys

import numpy as np

for _p in ("/opt/trn_rl_repo", "/root/.axon_site/_ro/trn_rl_repo"):
    if os.path.isdir(_p) and _p not in sys.path:
        sys.path.insert(0, _p)

import concourse.bass as bass
import concourse.mybir as mybir
import concourse.tile as tile
from concourse import bacc
from concourse.bass import ts, ds
from concourse.bass_utils import run_bass_kernel_spmd
from concourse.masks import make_identity

F32 = mybir.dt.float32
F32R = mybir.dt.float32r
F16 = mybir.dt.float16
I32 = mybir.dt.int32
MDT = F16 if os.environ.get("KMMDT", "f16") == "f16" else F32R
AF = mybir.ActivationFunctionType
OP = mybir.AluOpType

B, S, V, D, H, NL, MLPD = 2, 2048, 32000, 512, 8, 4, 2048
DH = D // H            # 64
HALF = 256             # attention half-window (WIN // 2)
P = 128
NCORES = 8
CHUNK = 512            # own tokens per core
W = CHUNK + HALF       # 768 = halo + own
DT = D // P            # 4 d-tiles
MT = MLPD // P         # 16 mlp tiles
VSH = 4096             # padded vocab shard (actual 4000)
NTOK = B * S           # 4096
VV = V // NCORES       # 4000 valid vocab per core
GROUPS = [[0, 1, 2, 3], [4, 5, 6, 7]]
EXP_SHIFT = 2.0
SCALE = float(1.0 / np.sqrt(DH))
# key-tile table: (kt, q0, width) — q columns [q0, q0+width) see key tile kt
KTW = [(0, 0, 256), (1, 0, 256), (2, 0, 512), (3, 0, 512), (4, 256, 256), (5, 256, 256)]

_CACHE = {}


# ================================================================ builder
def _build():
    nc = bacc.Bacc("TRN2", target_bir_lowering=False, debug=False,
                   num_devices=NCORES)

    ein = lambda n, sh, dt=F32: nc.dram_tensor(n, sh, dt, kind="ExternalInput")
    io = dict(
        wq=ein("wq", [NL, D, D], MDT), wk=ein("wk", [NL, D, D], MDT),
        wv=ein("wv", [NL, D, D], MDT), wo=ein("wo", [NL, D, D], MDT),
        w1=ein("w1", [NL, D, MLPD], MDT), w2=ein("w2", [NL, MLPD, D], MDT),
        b1=ein("b1", [NL, MLPD]), b2=ein("b2", [NL, D]),
        bq=ein("bq", [NL, D]), bk=ein("bk", [NL, D]), bv=ein("bv", [P, NL, D]),
        w_out_sl=ein("w_out_sl", [D, VSH], MDT), b_out_sl=ein("b_out_sl", [1, VSH]),
        embed=ein("embed", [V, D]),
        idx_in=ein("idx_in", [P, W // P], I32),
        pe_dm=ein("pe_dm", [D, W]),
        masks=ein("masks", [6, P, 512], F16),
        halo_offs=ein("halo_offs", [P, DT], I32),
        yoffs=ein("yoffs", [P, 7 * DT], I32),
        out=nc.dram_tensor("logits_vm", [VSH, NTOK], F16, kind="ExternalOutput"),
    )
    if os.environ.get("KDEBUG") == "1":
        io["xdump"] = nc.dram_tensor("xdump", [NL + 1, D, CHUNK], F32, kind="ExternalOutput")
        io["ydump"] = nc.dram_tensor("ydump", [D, W], MDT, kind="ExternalOutput")
        io["qdump"] = nc.dram_tensor("qdump", [D, CHUNK], MDT, kind="ExternalOutput")
        io["kdump"] = nc.dram_tensor("kdump", [D, W], MDT, kind="ExternalOutput")
        io["adump"] = nc.dram_tensor("adump", [D, CHUNK], MDT, kind="ExternalOutput")

    with tile.TileContext(nc) as tc, nc.allow_low_precision(reason="f32r rounding"):
        _emit(nc, tc, io)
    nc.compile()
    return nc


def _emit(nc, tc, io):
    cpool = tc.alloc_tile_pool(name="const", bufs=1)
    xpool = tc.alloc_tile_pool(name="xres", bufs=1)
    wqk = tc.alloc_tile_pool(name="wqkvo", bufs=2)
    wmlp = tc.alloc_tile_pool(name="wmlp", bufs=1)
    wofp = tc.alloc_tile_pool(name="wofp", bufs=1)
    ps_a = tc.alloc_tile_pool(name="ps_a", bufs=2, space="PSUM")
    ps_b = tc.alloc_tile_pool(name="ps_b", bufs=4, space="PSUM")
    ps_c = tc.alloc_tile_pool(name="ps_c", bufs=2, space="PSUM")
    drp = tc.alloc_tile_pool(name="drbounce", bufs=1, space="DRAM")

    # ------------------------------------------------ constants
    ones_f = cpool.tile([P, P], F32, tag="ones_f")
    nc.vector.memset(ones_f[:], 1.0)
    ones = cpool.tile([P, P], MDT, tag="ones")
    nc.vector.tensor_copy(out=ones[:], in_=ones_f[:])
    ones_r = cpool.tile([P, P], F32R, tag="ones_r")
    nc.vector.tensor_copy(out=ones_r[:], in_=ones_f[:])
    negb = cpool.tile([P, 1], F32, tag="negb")
    nc.vector.memset(negb[:], EXP_SHIFT)
    epsb = cpool.tile([P, 1], F32, tag="epsb")
    nc.vector.memset(epsb[:], 1e-6)
    b1t = cpool.tile([P, NL, MT], F32, tag="b1t")
    b2t = cpool.tile([P, NL, DT], F32, tag="b2t")
    bqt = cpool.tile([P, NL, DT], F32, tag="bqt")
    bkt = cpool.tile([P, NL, DT], F32, tag="bkt")
    bvt = cpool.tile([P, NL, D], F32, tag="bvt")
    bot = cpool.tile([P, VSH // P], F32, tag="bot")
    nc.sync.dma_start(out=b1t[:], in_=io["b1"].ap().rearrange("l (m p) -> p l m", p=P))
    nc.sync.dma_start(out=b2t[:], in_=io["b2"].ap().rearrange("l (t p) -> p l t", p=P))
    nc.sync.dma_start(out=bqt[:], in_=io["bq"].ap().rearrange("l (t p) -> p l t", p=P))
    nc.sync.dma_start(out=bkt[:], in_=io["bk"].ap().rearrange("l (t p) -> p l t", p=P))
    nc.sync.dma_start(out=bvt[:], in_=io["bv"].ap())
    nc.sync.dma_start(out=bot[:], in_=io["b_out_sl"].ap().rearrange("o (t p) -> p (o t)", p=P))
    maskt = cpool.tile([P, 6, 512], F16, tag="maskt")
    nc.sync.dma_start(out=maskt[:], in_=io["masks"].ap().rearrange("k p q -> p k q"))
    hoffs = cpool.tile([P, DT], I32, tag="hoffs")
    nc.sync.dma_start(out=hoffs[:], in_=io["halo_offs"].ap())
    yoffs = cpool.tile([P, 7 * DT], I32, tag="yoffs")
    nc.sync.dma_start(out=yoffs[:], in_=io["yoffs"].ap())

    # residual stream (own 512 tokens, d-major) + per-layer halo + final LN out.
    # F32R (same bits as f32) so the LN stat matmuls can consume x directly.
    x = xpool.tile([P, DT, CHUNK], F32R, tag="x")
    xh = xpool.tile([P, DT, HALF], F32R, tag="xh")
    yf = xpool.tile([P, DT, CHUNK], MDT, tag="yf")

    # ------------------------------------------------ weight loads
    def load_qkvo(l):
        w = {}
        for nm in ("wq", "wk", "wv", "wo"):
            t = wqk.tile([P, DT, D], MDT, tag=nm, name=nm)
            nc.sync.dma_start(out=t[:], in_=io[nm].ap()[l].rearrange("(t p) m -> p t m", p=P))
            w[nm] = t
        return w

    def load_mlp(l):
        w1r = wmlp.tile([P, DT, MLPD], MDT, tag="w1r")
        nc.sync.dma_start(out=w1r[:], in_=io["w1"].ap()[l].rearrange("(t p) m -> p t m", p=P))
        w2r = wmlp.tile([P, MT, D], MDT, tag="w2r")
        nc.sync.dma_start(out=w2r[:], in_=io["w2"].ap()[l].rearrange("(t p) m -> p t m", p=P))
        return w1r, w2r

    wcur = load_qkvo(0)
    mcur = load_mlp(0)

    # ------------------------------------------------ embedding
    with tc.tile_pool(name="embed", bufs=1) as epool:
        ident = epool.tile([P, P], F32, tag="ident")
        make_identity(nc, ident[:])
        pe = epool.tile([P, DT, W], F32, tag="pe")
        nc.sync.dma_start(out=pe[:], in_=io["pe_dm"].ap().rearrange("(t p) m -> p t m", p=P))
        idxt = epool.tile([P, W // P], I32, tag="idxt")
        nc.sync.dma_start(out=idxt[:], in_=io["idx_in"].ap())
        with tc.tile_pool(name="gath", bufs=2) as gpool:
            for g in range(W // P):
                gt = gpool.tile([P, D], F32, tag="gt")
                nc.gpsimd.indirect_dma_start(
                    out=gt[:], out_offset=None, in_=io["embed"].ap(),
                    in_offset=bass.IndirectOffsetOnAxis(ap=idxt[:, g:g + 1], axis=0),
                )
                for dt in range(DT):
                    pt = ps_a.tile([P, P], F32, tag="ps_a")
                    nc.tensor.transpose(pt[:], gt[:, ts(dt, P)], ident[:])
                    dst = xh[:, dt, ts(g, P)] if g < 2 else x[:, dt, ts(g - 2, P)]
                    nc.vector.tensor_add(out=dst, in0=pt[:], in1=pe[:, dt, ts(g, P)])

    def dump_x(slot):
        if "xdump" in io:
            nc.sync.dma_start(out=io["xdump"].ap()[slot].rearrange("(t p) m -> p t m", p=P),
                              in_=x[:, :, :].bitcast(F32))
    dump_x(0)

    # ------------------------------------------------ layer pools
    lp = tc.alloc_tile_pool(name="layers", bufs=1)
    lp3 = tc.alloc_tile_pool(name="ltrans3", bufs=3)

    def emit_ln(srcs, y):
        """LN over d (partition axis); emits z=(x-mu)*rstd (no affine).
        srcs: list of (fn(dt)->AP[128,width] F32, y_col0, width)."""
        srcs2 = []
        for fn, col0, width in srcs:
            for o in range(0, width, 256):
                srcs2.append((lambda dt, fn=fn, o=o: fn(dt)[:, ds(o, 256)], col0 + o))
        for fn, col0 in srcs2:
            sx = ps_c.tile([1, 256], F32, tag="ps_c")
            sxx = ps_c.tile([1, 256], F32, tag="ps_c")
            for dt in range(DT):
                xsq = lp3.tile([P, 256], F32R, tag="ln_xsq", bufs=2)
                nc.vector.tensor_mul(out=xsq[:], in0=fn(dt), in1=fn(dt))
                nc.tensor.matmul(out=sx[:], lhsT=ones_r[:, 0:1], rhs=fn(dt),
                                 start=(dt == 0), stop=(dt == DT - 1))
                nc.tensor.matmul(out=sxx[:], lhsT=ones_r[:, 0:1], rhs=xsq[:],
                                 start=(dt == 0), stop=(dt == DT - 1))
            mu = lp3.tile([1, 256], F32R, tag="ln_mu", bufs=2)
            mu2 = lp3.tile([1, 256], F32, tag="ln_mu2", bufs=2)
            var = lp3.tile([1, 256], F32, tag="ln_var", bufs=2)
            sd = lp3.tile([1, 256], F32, tag="ln_sd", bufs=2)
            rstd = lp3.tile([1, 256], F32, tag="ln_rstd", bufs=2)
            nc.vector.tensor_scalar_mul(out=mu[:], in0=sx[:], scalar1=1.0 / D)
            nc.vector.tensor_mul(out=mu2[:], in0=mu[:], in1=mu[:])
            # var = sxx/D - mu^2
            nc.vector.scalar_tensor_tensor(
                out=var[:], in0=sxx[:], scalar=1.0 / D,
                in1=mu2[:], op0=OP.mult, op1=OP.subtract)
            nc.scalar.activation(sd[:], var[:], AF.Sqrt, bias=epsb[0:1, :], scale=1.0)
            nc.vector.reciprocal_approx_fast(out=rstd[:], in_=sd[:])
            rstd16 = lp3.tile([1, 256], MDT, tag="ln_rstd16", bufs=2)
            nc.scalar.copy(out=rstd16[:], in_=rstd[:])
            pmu = ps_a.tile([P, 256], F32, tag="ps_a")
            nc.tensor.matmul(out=pmu[:], lhsT=ones_r[0:1, :], rhs=mu[:],
                             start=True, stop=True)
            prs = ps_a.tile([P, 256], F32, tag="ps_a")
            nc.tensor.matmul(out=prs[:], lhsT=ones[0:1, :], rhs=rstd16[:],
                             start=True, stop=True)
            for dt in range(DT):
                scr = lp3.tile([P, 256], F32, tag="ln_scr", bufs=2)
                nc.vector.tensor_sub(out=scr[:], in0=fn(dt), in1=pmu[:])
                nc.vector.tensor_mul(out=y[:, dt, ds(col0, 256)], in0=scr[:], in1=prs[:])

    # ------------------------------------------------ transformer layers
    _knl = int(os.environ.get("KNL", NL))
    for l in range(_knl):
        wq_r, wk_r, wv_r, wo_r = wcur["wq"], wcur["wk"], wcur["wv"], wcur["wo"]
        w1r, w2r = mcur

        y = lp.tile([P, DT, W], MDT, tag="y")
        # LN1 on own tokens (halo part deferred until the AllGather landed)
        emit_ln(srcs=[(lambda dt: x[:, dt, :], HALF, CHUNK)], y=y)

        # --- Q projection (own tokens only)
        qr = lp.tile([P, DT, CHUNK], MDT, tag="qr")
        for do in range(DT):
            pq = ps_a.tile([P, CHUNK], F32, tag="ps_a")
            for dt in range(DT):
                nc.tensor.matmul(out=pq[:], lhsT=wq_r[:, dt, ts(do, P)],
                                 rhs=y[:, dt, ds(HALF, CHUNK)],
                                 start=(dt == 0), stop=(dt == DT - 1))
            nc.scalar.activation(qr[:, do, :], pq[:], AF.Identity,
                                 bias=bqt[:, l, do:do + 1], scale=1.0)

        # --- K/V projections, own tokens
        kr = lp.tile([P, DT, W], MDT, tag="kr")
        for do in range(DT):
            pk = ps_a.tile([P, CHUNK], F32, tag="ps_a")
            for dt in range(DT):
                nc.tensor.matmul(out=pk[:], lhsT=wk_r[:, dt, ts(do, P)],
                                 rhs=y[:, dt, ds(HALF, CHUNK)],
                                 start=(dt == 0), stop=(dt == DT - 1))
            nc.scalar.activation(kr[:, do, ds(HALF, CHUNK)], pk[:], AF.Identity,
                                 bias=bkt[:, l, do:do + 1], scale=1.0)

        vt = [lp.tile([P, H * (DH + 1)], MDT, tag=f"vt{t}", name=f"vt{t}") for t in range(W // P)]

        def emit_v(t, ysrc):
            pv = ps_a.tile([P, D], F32, tag="ps_a")
            for dt in range(DT):
                nc.tensor.matmul(out=pv[:], lhsT=ysrc(dt, t), rhs=wv_r[:, dt, :],
                                 start=(dt == 0), stop=(dt == DT - 1))
            vtv = vt[t][:].rearrange("p (h c) -> p h c", c=DH + 1)
            nc.vector.tensor_add(
                out=vtv[:, :, 0:DH],
                in0=pv[:].rearrange("p (h c) -> p h c", c=DH),
                in1=bvt[:, l, :].rearrange("p (h c) -> p h c", c=DH))
            nc.vector.tensor_copy(out=vtv[:, :, DH:DH + 1], in_=ones[:, 0:H])

        for t in range(2, W // P):
            emit_v(t, lambda dt, t: y[:, dt, ts(t, P)])

        # --- sliding-window attention
        attr = lp.tile([P, DT, CHUNK], MDT, tag="attr")
        ej_keep = {}

        def emit_ej(h, kt, keep):
            _, q0, w = KTW[kt]
            r0, dto = (h % 2) * DH, h // 2
            pscore = ps_b.tile([P, 512], F32, tag="ps_b")
            nc.tensor.matmul(out=pscore[:, 0:w],
                             lhsT=kr[ds(r0, DH), dto, ts(kt, P)],
                             rhs=qr[ds(r0, DH), dto, ds(q0, w)],
                             start=True, stop=True)
            ej = lp3.tile([P, 512], MDT, tag="ej_keep" if keep else "ej_tmp",
                          bufs=2 * H if keep else 4, name="ej")
            nc.scalar.activation(ej[:, 0:w], pscore[:, 0:w], AF.Exp,
                                 bias=negb[:], scale=SCALE)
            nc.vector.tensor_mul(out=ej[:, 0:w], in0=ej[:, 0:w], in1=maskt[:, kt, 0:w])
            return ej

        def emit_qblock(h, qB, ejs):
            r0, dto = (h % 2) * DH, h // 2
            pa = ps_c.tile([DH + 1, 256], F32, tag="ps_c")
            for i, kt in enumerate(range(qB * 2, qB * 2 + 4)):
                c0 = qB * 256 - KTW[kt][1]
                nc.tensor.matmul(out=pa[:], lhsT=vt[kt][:, ds(h * (DH + 1), DH + 1)],
                                 rhs=ejs[kt][:, ds(c0, 256)],
                                 start=(i == 0), stop=(i == 3))
            # reciprocal_approx_* misreads PSUM at base_partition!=0 — stage
            # the denominator row to a partition-0 SBUF tile first.
            srow = lp3.tile([1, 256], F32, tag="srow", bufs=2)
            nc.scalar.copy(out=srow[:], in_=pa[DH:DH + 1, :])
            rr = lp3.tile([1, 256], F32, tag="rr", bufs=2)
            nc.vector.reciprocal_approx_fast(out=rr[:], in_=srow[:])
            rr16 = lp3.tile([1, 256], MDT, tag="rr16", bufs=2)
            nc.scalar.copy(out=rr16[:], in_=rr[:])
            pbc = ps_a.tile([DH, 256], F32, tag="ps_a")
            nc.tensor.matmul(out=pbc[:], lhsT=ones[0:1, 0:DH], rhs=rr16[:],
                             start=True, stop=True)
            dst = attr[ds(r0, DH), dto, ds(qB * 256, 256)]
            bcs = lp3.tile([DH, 256], MDT, tag="bcs", bufs=3)
            nc.scalar.copy(out=bcs[:], in_=pbc[:])
            nc.vector.tensor_mul(out=dst, in0=pa[0:DH, :], in1=bcs[:])

        # phase 1: own-key work for all heads (kt 2..5, qB=1)
        for h in range(H):
            ejs = {kt: emit_ej(h, kt, kt in (2, 3)) for kt in (2, 3, 4, 5)}
            ej_keep[h] = {kt: ejs[kt] for kt in (2, 3)}
            emit_qblock(h, 1, ejs)

        # halo now: LN1 on xh, K/V halo columns (consumes prev layer's AllGather)
        emit_ln(srcs=[(lambda dt: xh[:, dt, :], 0, HALF)], y=y)
        for do in range(DT):
            pk = ps_a.tile([P, HALF], F32, tag="ps_a")
            for dt in range(DT):
                nc.tensor.matmul(out=pk[:], lhsT=wk_r[:, dt, ts(do, P)],
                                 rhs=y[:, dt, ds(0, HALF)],
                                 start=(dt == 0), stop=(dt == DT - 1))
            nc.scalar.activation(kr[:, do, ds(0, HALF)], pk[:], AF.Identity,
                                 bias=bkt[:, l, do:do + 1], scale=1.0)
        for t in range(2):
            emit_v(t, lambda dt, t: y[:, dt, ts(t, P)])

        # phase 2: halo-key work (kt 0,1 + kept kt 2,3; qB=0)
        _rekt = os.environ.get("KREKT") == "1"
        for h in range(H):
            ejs = dict(ej_keep[h])
            for kt in (0, 1):
                ejs[kt] = emit_ej(h, kt, False)
            if _rekt:
                for kt in (2, 3):
                    ejs[kt] = emit_ej(h, kt, False)
            emit_qblock(h, 0, ejs)

        if l == 0 and "ydump" in io:
            for nm_t, src in (("ydump", y), ("qdump", qr), ("kdump", kr), ("adump", attr)):
                nc.sync.dma_start(out=io[nm_t].ap().rearrange("(t p) m -> p t m", p=P),
                                  in_=src[:])

        # --- output projection + residual
        for do in range(DT):
            po = ps_a.tile([P, CHUNK], F32, tag="ps_a")
            for dt in range(DT):
                nc.tensor.matmul(out=po[:], lhsT=wo_r[:, dt, ts(do, P)],
                                 rhs=attr[:, dt, :], start=(dt == 0), stop=(dt == DT - 1))
            nc.vector.tensor_add(out=x[:, do, :], in0=x[:, do, :], in1=po[:])

        # prefetch next layer's attention weights (double-buffered pool)
        if l + 1 < _knl:
            wcur = load_qkvo(l + 1)

        # --- LN2 + MLP
        y2 = lp.tile([P, DT, CHUNK], MDT, tag="y2")
        emit_ln(srcs=[(lambda dt: x[:, dt, :], 0, CHUNK)], y=y2)

        pb = [ps_b.tile([P, CHUNK], F32, tag="ps_b", name=f"pb{i}") for i in range(DT)]

        def emit_mlp2(m, hm):
            for do in range(DT):
                nc.tensor.matmul(out=pb[do][:], lhsT=w2r[:, m, ts(do, P)],
                                 rhs=hm[:], start=(m == 0), stop=(m == MT - 1))

        hist = []
        for m in range(MT):
            p1 = ps_a.tile([P, CHUNK], F32, tag="ps_a")
            for dt in range(DT):
                nc.tensor.matmul(out=p1[:], lhsT=w1r[:, dt, ts(m, P)],
                                 rhs=y2[:, dt, :],
                                 start=(dt == 0), stop=(dt == DT - 1))
            hm = lp3.tile([P, CHUNK], MDT, tag="hm", bufs=3)
            nc.scalar.activation(hm[:], p1[:], AF.Gelu_apprx_tanh,
                                 bias=b1t[:, l, m:m + 1], scale=1.0)
            hist.append((m, hm))
            if len(hist) > 2:
                emit_mlp2(*hist.pop(0))
        for mm_, hh_ in hist:
            emit_mlp2(mm_, hh_)

        # prefetch next layer's MLP weights (single buffer: reallocates after use)
        if l + 1 < _knl:
            mcur = load_mlp(l + 1)

        # residual (+b2), then send halo for next layer
        for do in range(DT):
            nc.vector.scalar_tensor_tensor(
                out=x[:, do, :], in0=pb[do][:],
                scalar=b2t[:, l, do:do + 1], in1=x[:, do, :],
                op0=OP.add, op1=OP.add)
        if l < NL - 1:
            agin = drp.tile([D, HALF], F32R, tag=f"agin{l}")
            agout = drp.tile([len(GROUPS[0]) * D, HALF], F32R, tag=f"agout{l}")
            nc.sync.dma_start(out=agin[:].rearrange("(t p) m -> p t m", p=P),
                              in_=x[:, :, ds(HALF, HALF)])
            nc.gpsimd.collective_compute(
                "AllGather", OP.bypass, replica_groups=GROUPS,
                ins=[agin.opt()], outs=[agout.opt()])
            for dt in range(DT):
                nc.gpsimd.indirect_dma_start(
                    out=xh[:, dt, :], out_offset=None, in_=agout[:],
                    in_offset=bass.IndirectOffsetOnAxis(ap=hoffs[:, dt:dt + 1], axis=0))
        dump_x(l + 1)

    # ------------------------------------------------ final LN + allgather
    emit_ln(srcs=[(lambda dt: x[:, dt, :], 0, CHUNK)], y=yf)
    yfd = drp.tile([D, CHUNK], MDT, tag="yfd")
    nc.sync.dma_start(out=yfd[:].rearrange("(t p) m -> p t m", p=P), in_=yf[:])
    yfg = drp.tile([NCORES * D, CHUNK], MDT, tag="yfg", addr_space="Shared")
    nc.gpsimd.collective_compute(
        "AllGather", OP.bypass, replica_groups=[list(range(NCORES))],
        ins=[yfd.opt()], outs=[yfg.opt()])

    # w_out cache: issued here, but the sync queue reaches it during the last
    # layer's compute, so the 4MB load overlaps.
    wof = wofp.tile([P, DT, VSH], MDT, tag="wof")
    nc.sync.dma_start(out=wof[:], in_=io["w_out_sl"].ap().rearrange("(t p) m -> p t m", p=P))

    lp3.release()
    lp.release()

    # ------------------------------------------------ vocab-sharded logits
    # slot 0 = own tokens (local yf, overlaps the AllGather); slot j>0 = core
    # (c+j)%8's tokens, imported via per-core indirect offsets.
    with tc.tile_pool(name="final", bufs=1) as fpool, \
         tc.tile_pool(name="ftrans", bufs=4) as ftp:
        yall = fpool.tile([P, 7, DT, CHUNK], MDT, tag="yall")

        def emit_slot(j, rhs_of):
            for v_i in range(VSH // P):
                pf = ps_a.tile([P, CHUNK], F32, tag="ps_a")
                for dt in range(DT):
                    nc.tensor.matmul(out=pf[:], lhsT=wof[:, dt, ts(v_i, P)],
                                     rhs=rhs_of(dt), start=(dt == 0), stop=(dt == DT - 1))
                ot = ftp.tile([P, CHUNK], F16, tag="fot")
                if v_i % 2 == 0:
                    nc.scalar.activation(ot[:], pf[:], AF.Identity,
                                         bias=bot[:, v_i:v_i + 1], scale=1.0)
                else:
                    nc.vector.tensor_scalar_add(out=ot[:], in0=pf[:],
                                                scalar1=bot[:, v_i:v_i + 1])
                nc.sync.dma_start(out=io["out"].ap()[ts(v_i, P), ts(j, CHUNK)],
                                  in_=ot[:])

        emit_slot(0, lambda dt: yf[:, dt, :])
        for j in range(1, NCORES):
            for dt in range(DT):
                nc.gpsimd.indirect_dma_start(
                    out=yall[:, j - 1, dt, :], out_offset=None, in_=yfg[:],
                    in_offset=bass.IndirectOffsetOnAxis(
                        ap=yoffs[:, (j - 1) * DT + dt:(j - 1) * DT + dt + 1], axis=0))
            emit_slot(j, lambda dt, j=j: yall[:, j - 1, dt, :])

    wofp.release()
    drp.release()
    ps_c.release()
    ps_b.release()
    ps_a.release()
    wmlp.release()
    wqk.release()
    xpool.release()
    cpool.release()


# ================================================================ host side
def _pe_table():
    pos = np.arange(S, dtype=np.float32)[:, None]
    div = np.exp(np.arange(0, D, 2, dtype=np.float32) * -(np.log(10000.0) / D))
    pe = np.zeros((S, D), dtype=np.float32)
    pe[:, 0::2] = np.sin(pos * div)
    pe[:, 1::2] = np.cos(pos * div)
    return pe


def _in_maps(inputs):
    inp = np.asarray(inputs["inputs"]).astype(np.int32)
    ids = np.pad(inp, ((0, 0), (1, 0)))[:, :-1].astype(np.int32)
    pe = _pe_table()

    f32 = lambda k: np.asarray(inputs[k], dtype=np.float32)
    ln1_s, ln1_b = f32("ln1_s"), f32("ln1_b")
    ln2_s, ln2_b = f32("ln2_s"), f32("ln2_b")
    lnf_s, lnf_b = f32("lnf_s").reshape(D), f32("lnf_b").reshape(D)
    wq, wk, wv, wo = f32("wq"), f32("wk"), f32("wv"), f32("wo")
    w1, w2 = f32("w1"), f32("w2")
    b1, b2 = f32("b1"), f32("b2")
    wout, bout = f32("w_out"), f32("b_out")

    # fold LN affine into the downstream projections
    wq_f = wq * ln1_s[:, :, None]
    wk_f = wk * ln1_s[:, :, None]
    wv_f = wv * ln1_s[:, :, None]
    w1_f = w1 * ln2_s[:, :, None]
    bq = np.einsum("ld,ldm->lm", ln1_b, wq)
    bk = np.einsum("ld,ldm->lm", ln1_b, wk)
    bv = np.einsum("ld,ldm->lm", ln1_b, wv)
    b1_f = b1 + np.einsum("ld,ldm->lm", ln2_b, w1)
    wout_f = wout * lnf_s[:, None]
    bout_f = bout + lnf_b @ wout

    shared = {
        "embed": np.ascontiguousarray(f32("embed")),
        "b1": b1_f, "b2": b2, "bq": bq, "bk": bk,
        "bv": np.ascontiguousarray(np.broadcast_to(bv[None], (P, NL, D))),
        "wq": wq_f.astype(np.float16), "wk": wk_f.astype(np.float16),
        "wv": wv_f.astype(np.float16), "wo": wo.astype(np.float16),
        "w1": w1_f.astype(np.float16), "w2": w2.astype(np.float16),
    }
    shared = {k: np.ascontiguousarray(v) for k, v in shared.items()}
    wout16 = wout_f.astype(np.float16)

    maps = []
    for c in range(NCORES):
        b, ch = divmod(c, NCORES // B)
        t0 = ch * CHUNK
        lo = t0 - HALF
        ids768 = np.zeros(W, np.int32)
        pe768 = np.zeros((W, D), np.float32)
        s0 = max(0, lo)
        ids768[s0 - lo:] = ids[b, s0:t0 + CHUNK]
        pe768[s0 - lo:] = pe[s0:t0 + CHUNK]
        # per-key-tile masks: [6, 128, 512] f16
        m = np.zeros((6, P, 512), np.float16)
        for kt, q0, w in KTW:
            uk = kt * P + np.arange(P)[:, None]
            q = q0 + np.arange(w)[None, :]
            dqk = (HALF + q) - uk
            ok = (dqk >= 0) & (dqk <= HALF)
            if ch == 0:
                ok = ok & ((lo + uk) >= 0)
            m[kt, :, :w] = ok.astype(np.float16)
        src = ch - 1 if ch > 0 else 0
        hoffs = (src * D + np.arange(DT)[None, :] * P
                 + np.arange(P)[:, None]).astype(np.int32)
        yo = np.zeros((P, 7 * DT), np.int32)
        for j in range(1, NCORES):
            sc = (c + j) % NCORES
            for dt in range(DT):
                yo[:, (j - 1) * DT + dt] = sc * D + dt * P + np.arange(P)
        vlo = c * VV
        wsl = np.zeros((D, VSH), np.float16)
        wsl[:, :VV] = wout16[:, vlo:vlo + VV]
        bsl = np.zeros((1, VSH), np.float32)
        bsl[0, :VV] = bout_f[vlo:vlo + VV]
        mp = dict(shared)
        mp.update(
            idx_in=np.ascontiguousarray(ids768.reshape(W // P, P).T),
            pe_dm=np.ascontiguousarray(pe768.T),
            masks=m, halo_offs=hoffs, yoffs=yo, w_out_sl=wsl, b_out_sl=bsl)
        maps.append(mp)
    return maps


def _assemble(res):
    full = np.empty((NTOK, V), np.float32)
    for c in range(NCORES):
        lv = np.asarray(res[c]["logits_vm"], dtype=np.float32)  # [VSH, NTOK] rotated
        for j in range(NCORES):
            blk = (c + j) % NCORES
            full[blk * CHUNK:(blk + 1) * CHUNK, c * VV:(c + 1) * VV] = \
                lv[:VV, j * CHUNK:(j + 1) * CHUNK].T
    return full.reshape(B, S, V)


def kernel(**inputs):
    nc = _CACHE.get("nc")
    if nc is None:
        nc = _build()
        _CACHE["nc"] = nc
    maps = _in_maps(inputs)
    res = run_bass_kernel_spmd(nc, maps, list(range(NCORES))).results
    return _assemble(res)


# revision 22
# speedup vs baseline: 1.3360x; 1.0302x over previous
"""Longformer decoder (4 layers, sliding-window causal attention) on 8 trn2 cores.

Sharding: 4096 tokens (B=2 x S=2048) split into 8 contiguous chunks of 512
(core = b*4 + chunk). Activations are kept d-major ([dim, token], dim on
partitions) so every matmul is weights-stationary with no transposes.

v2 structure (vs v1 baseline):
- LN affine (scale/bias) folded into the projection weights host-side; the
  kernel's LN emits the unaffined z=(x-mu)*rstd, with biases re-applied via
  scalar-engine Identity copies (per-partition bias) where needed.
- LN statistics matmuls run on f32r bitcasts of the residual directly (no
  gpsimd f16 staging copies).
- reciprocal_approx_fast for all softmax/LN reciprocals.
- Attention restructured per-head into kt (key-tile) granularity with f16
  masks; per layer, all heads' own-key work (qB=1) runs first so the x-halo
  AllGather from the previous layer is consumed ~40us into the layer.
- Weights double-buffered and prefetched one layer ahead; w_out cached in
  SBUF during the last layer.
- Final logits loop runs the core's own 512-token block first (directly off
  the local yf) while the 8-way AllGather is in flight; remaining blocks are
  imported with per-core indirect DMAs. Output token-blocks are rotated
  per-core ((c+j)%8) so the SPMD program stays identical; host unrotates.
- Logits written f16 (tolerance 2e-2; f16 quantization ~5e-4).
"""
import os
import sys

import numpy as np

for _p in ("/opt/trn_rl_repo", "/root/.axon_site/_ro/trn_rl_repo"):
    if os.path.isdir(_p) and _p not in sys.path:
        sys.path.insert(0, _p)

import concourse.bass as bass
import concourse.mybir as mybir
import concourse.tile as tile
from concourse import bacc
from concourse.bass import ts, ds
from concourse.bass_utils import run_bass_kernel_spmd
from concourse.masks import make_identity

F32 = mybir.dt.float32
F32R = mybir.dt.float32r
F16 = mybir.dt.float16
I32 = mybir.dt.int32
MDT = F16 if os.environ.get("KMMDT", "f16") == "f16" else F32R
AF = mybir.ActivationFunctionType
OP = mybir.AluOpType

B, S, V, D, H, NL, MLPD = 2, 2048, 32000, 512, 8, 4, 2048
DH = D // H            # 64
HALF = 256             # attention half-window (WIN // 2)
P = 128
NCORES = 8
CHUNK = 512            # own tokens per core
W = CHUNK + HALF       # 768 = halo + own
DT = D // P            # 4 d-tiles
MT = MLPD // P         # 16 mlp tiles
VSH = 4096             # padded vocab shard (actual 4000)
NTOK = B * S           # 4096
VV = V // NCORES       # 4000 valid vocab per core
GROUPS = [[0, 1, 2, 3], [4, 5, 6, 7]]
EXP_SHIFT = 2.0
SCALE = float(1.0 / np.sqrt(DH))
# key-tile table: (kt, q0, width) — q columns [q0, q0+width) see key tile kt
KTW = [(0, 0, 256), (1, 0, 256), (2, 0, 512), (3, 0, 512), (4, 256, 256), (5, 256, 256)]

_CACHE = {}


# ================================================================ builder
def _build():
    nc = bacc.Bacc("TRN2", target_bir_lowering=False, debug=False,
                   num_devices=NCORES)

    ein = lambda n, sh, dt=F32: nc.dram_tensor(n, sh, dt, kind="ExternalInput")
    io = dict(
        wq=ein("wq", [NL, D, D], MDT), wk=ein("wk", [NL, D, D], MDT),
        wv=ein("wv", [NL, D, D], MDT), wo=ein("wo", [NL, D, D], MDT),
        w1=ein("w1", [NL, D, MLPD], MDT), w2=ein("w2", [NL, MLPD, D], MDT),
        b1=ein("b1", [NL, MLPD]), b2=ein("b2", [NL, D]),
        bq=ein("bq", [NL, D]), bk=ein("bk", [NL, D]), bv=ein("bv", [P, NL, D]),
        w_out_sl=ein("w_out_sl", [D, VSH], MDT), b_out_sl=ein("b_out_sl", [1, VSH]),
        embed=ein("embed", [V, D]),
        idx_in=ein("idx_in", [P, W // P], I32),
        pe_dm=ein("pe_dm", [D, W]),
        masks=ein("masks", [6, P, 512], F16),
        halo_offs=ein("halo_offs", [P, DT], I32),
        yoffs=ein("yoffs", [P, 7 * DT], I32),
        out=nc.dram_tensor("logits_vm", [VSH, NTOK], F16, kind="ExternalOutput"),
    )
    if os.environ.get("KDEBUG") == "1":
        io["xdump"] = nc.dram_tensor("xdump", [NL + 1, D, CHUNK], F32, kind="ExternalOutput")
        io["ydump"] = nc.dram_tensor("ydump", [D, W], MDT, kind="ExternalOutput")
        io["qdump"] = nc.dram_tensor("qdump", [D, CHUNK], MDT, kind="ExternalOutput")
        io["kdump"] = nc.dram_tensor("kdump", [D, W], MDT, kind="ExternalOutput")
        io["adump"] = nc.dram_tensor("adump", [D, CHUNK], MDT, kind="ExternalOutput")

    with tile.TileContext(nc) as tc, nc.allow_low_precision(reason="f32r rounding"):
        _emit(nc, tc, io)
    nc.compile()
    return nc


def _emit(nc, tc, io):
    cpool = tc.alloc_tile_pool(name="const", bufs=1)
    xpool = tc.alloc_tile_pool(name="xres", bufs=1)
    wqk = tc.alloc_tile_pool(name="wqkvo", bufs=2)
    wmlp = tc.alloc_tile_pool(name="wmlp", bufs=1)
    wofp = tc.alloc_tile_pool(name="wofp", bufs=1)
    ps_a = tc.alloc_tile_pool(name="ps_a", bufs=2, space="PSUM")
    ps_b = tc.alloc_tile_pool(name="ps_b", bufs=4, space="PSUM")
    ps_c = tc.alloc_tile_pool(name="ps_c", bufs=2, space="PSUM")
    drp = tc.alloc_tile_pool(name="drbounce", bufs=1, space="DRAM")

    # ------------------------------------------------ constants
    ones_f = cpool.tile([P, P], F32, tag="ones_f")
    nc.vector.memset(ones_f[:], 1.0)
    ones = cpool.tile([P, P], MDT, tag="ones")
    nc.vector.tensor_copy(out=ones[:], in_=ones_f[:])
    ones_r = cpool.tile([P, P], F32R, tag="ones_r")
    nc.vector.tensor_copy(out=ones_r[:], in_=ones_f[:])
    negb = cpool.tile([P, 1], F32, tag="negb")
    nc.vector.memset(negb[:], EXP_SHIFT)
    epsb = cpool.tile([P, 1], F32, tag="epsb")
    nc.vector.memset(epsb[:], 1e-6)
    b1t = cpool.tile([P, NL, MT], F32, tag="b1t")
    b2t = cpool.tile([P, NL, DT], F32, tag="b2t")
    bqt = cpool.tile([P, NL, DT], F32, tag="bqt")
    bkt = cpool.tile([P, NL, DT], F32, tag="bkt")
    bvt = cpool.tile([P, NL, D], F32, tag="bvt")
    bot = cpool.tile([P, VSH // P], F32, tag="bot")
    nc.sync.dma_start(out=b1t[:], in_=io["b1"].ap().rearrange("l (m p) -> p l m", p=P))
    nc.sync.dma_start(out=b2t[:], in_=io["b2"].ap().rearrange("l (t p) -> p l t", p=P))
    nc.sync.dma_start(out=bqt[:], in_=io["bq"].ap().rearrange("l (t p) -> p l t", p=P))
    nc.sync.dma_start(out=bkt[:], in_=io["bk"].ap().rearrange("l (t p) -> p l t", p=P))
    nc.sync.dma_start(out=bvt[:], in_=io["bv"].ap())
    nc.sync.dma_start(out=bot[:], in_=io["b_out_sl"].ap().rearrange("o (t p) -> p (o t)", p=P))
    maskt = cpool.tile([P, 6, 512], F16, tag="maskt")
    nc.sync.dma_start(out=maskt[:], in_=io["masks"].ap().rearrange("k p q -> p k q"))
    hoffs = cpool.tile([P, DT], I32, tag="hoffs")
    nc.sync.dma_start(out=hoffs[:], in_=io["halo_offs"].ap())
    yoffs = cpool.tile([P, 7 * DT], I32, tag="yoffs")
    nc.sync.dma_start(out=yoffs[:], in_=io["yoffs"].ap())

    # residual stream (own 512 tokens, d-major) + per-layer halo + final LN out.
    # F32R (same bits as f32) so the LN stat matmuls can consume x directly.
    x = xpool.tile([P, DT, CHUNK], F32R, tag="x")
    xh = xpool.tile([P, DT, HALF], F32R, tag="xh")
    yf = xpool.tile([P, DT, CHUNK], MDT, tag="yf")

    # ------------------------------------------------ weight loads
    def load_qkvo(l):
        w = {}
        for nm in ("wq", "wk", "wv", "wo"):
            t = wqk.tile([P, DT, D], MDT, tag=nm, name=nm)
            nc.sync.dma_start(out=t[:], in_=io[nm].ap()[l].rearrange("(t p) m -> p t m", p=P))
            w[nm] = t
        return w

    def load_mlp(l):
        w1r = wmlp.tile([P, DT, MLPD], MDT, tag="w1r")
        nc.sync.dma_start(out=w1r[:], in_=io["w1"].ap()[l].rearrange("(t p) m -> p t m", p=P))
        w2r = wmlp.tile([P, MT, D], MDT, tag="w2r")
        nc.sync.dma_start(out=w2r[:], in_=io["w2"].ap()[l].rearrange("(t p) m -> p t m", p=P))
        return w1r, w2r

    wcur = load_qkvo(0)
    mcur = load_mlp(0)

    # ------------------------------------------------ embedding
    with tc.tile_pool(name="embed", bufs=1) as epool:
        ident = epool.tile([P, P], F32, tag="ident")
        make_identity(nc, ident[:])
        pe = epool.tile([P, DT, W], F32, tag="pe")
        nc.sync.dma_start(out=pe[:], in_=io["pe_dm"].ap().rearrange("(t p) m -> p t m", p=P))
        idxt = epool.tile([P, W // P], I32, tag="idxt")
        nc.sync.dma_start(out=idxt[:], in_=io["idx_in"].ap())
        with tc.tile_pool(name="gath", bufs=2) as gpool:
            for g in range(W // P):
                gt = gpool.tile([P, D], F32, tag="gt")
                nc.gpsimd.indirect_dma_start(
                    out=gt[:], out_offset=None, in_=io["embed"].ap(),
                    in_offset=bass.IndirectOffsetOnAxis(ap=idxt[:, g:g + 1], axis=0),
                )
                for dt in range(DT):
                    pt = ps_a.tile([P, P], F32, tag="ps_a")
                    nc.tensor.transpose(pt[:], gt[:, ts(dt, P)], ident[:])
                    dst = xh[:, dt, ts(g, P)] if g < 2 else x[:, dt, ts(g - 2, P)]
                    nc.vector.tensor_add(out=dst, in0=pt[:], in1=pe[:, dt, ts(g, P)])

    def dump_x(slot):
        if "xdump" in io:
            nc.sync.dma_start(out=io["xdump"].ap()[slot].rearrange("(t p) m -> p t m", p=P),
                              in_=x[:, :, :].bitcast(F32))
    dump_x(0)

    # ------------------------------------------------ layer pools
    lp = tc.alloc_tile_pool(name="layers", bufs=1)
    lp3 = tc.alloc_tile_pool(name="ltrans3", bufs=3)

    def emit_ln(srcs, y):
        """LN over d (partition axis); emits z=(x-mu)*rstd (no affine).
        srcs: list of (fn(dt)->AP[128,width] F32, y_col0, width)."""
        srcs2 = []
        for fn, col0, width in srcs:
            for o in range(0, width, 256):
                srcs2.append((lambda dt, fn=fn, o=o: fn(dt)[:, ds(o, 256)], col0 + o))
        for fn, col0 in srcs2:
            sx = ps_c.tile([1, 256], F32, tag="ps_c")
            sxx = ps_c.tile([1, 256], F32, tag="ps_c")
            for dt in range(DT):
                xsq = lp3.tile([P, 256], F32R, tag="ln_xsq", bufs=2)
                nc.vector.tensor_mul(out=xsq[:], in0=fn(dt), in1=fn(dt))
                nc.tensor.matmul(out=sx[:], lhsT=ones_r[:, 0:1], rhs=fn(dt),
                                 start=(dt == 0), stop=(dt == DT - 1))
                nc.tensor.matmul(out=sxx[:], lhsT=ones_r[:, 0:1], rhs=xsq[:],
                                 start=(dt == 0), stop=(dt == DT - 1))
            mu = lp3.tile([1, 256], F32R, tag="ln_mu", bufs=2)
            mu2 = lp3.tile([1, 256], F32, tag="ln_mu2", bufs=2)
            var = lp3.tile([1, 256], F32, tag="ln_var", bufs=2)
            sd = lp3.tile([1, 256], F32, tag="ln_sd", bufs=2)
            rstd = lp3.tile([1, 256], F32, tag="ln_rstd", bufs=2)
            nc.vector.tensor_scalar_mul(out=mu[:], in0=sx[:], scalar1=1.0 / D)
            nc.vector.tensor_mul(out=mu2[:], in0=mu[:], in1=mu[:])
            # var = sxx/D - mu^2
            nc.vector.scalar_tensor_tensor(
                out=var[:], in0=sxx[:], scalar=1.0 / D,
                in1=mu2[:], op0=OP.mult, op1=OP.subtract)
            nc.scalar.activation(sd[:], var[:], AF.Sqrt, bias=epsb[0:1, :], scale=1.0)
            nc.vector.reciprocal_approx_fast(out=rstd[:], in_=sd[:])
            # broadcast mu (cols 0:256) and rstd (cols 256:512) in one matmul
            murs = lp3.tile([1, 512], MDT, tag="ln_murs", bufs=2)
            nc.scalar.copy(out=murs[:, 0:256], in_=mu[:])
            nc.scalar.copy(out=murs[:, 256:512], in_=rstd[:])
            pmr = ps_a.tile([P, 512], F32, tag="ps_a")
            nc.tensor.matmul(out=pmr[:], lhsT=ones[0:1, :], rhs=murs[:],
                             start=True, stop=True)
            for dt in range(DT):
                scr = lp3.tile([P, 256], F32, tag="ln_scr", bufs=2)
                nc.vector.tensor_sub(out=scr[:], in0=fn(dt), in1=pmr[:, 0:256])
                nc.vector.tensor_mul(out=y[:, dt, ds(col0, 256)], in0=scr[:], in1=pmr[:, 256:512])

    # ------------------------------------------------ transformer layers
    # Each layer is split into token-halves: the second half (tokens 256-511)
    # finishes first — through attention qB=1, O-proj, MLP, residual — and its
    # x is exported + AllGathered mid-layer. The gather-dependent first half
    # (halo LN, K/V halo, qB=0) of the NEXT layer then trails the export by a
    # full half-layer of independent work, hiding collective latency + skew.
    _knl = int(os.environ.get("KNL", NL))
    prev_agout = None
    for l in range(_knl):
        wq_r, wk_r, wv_r, wo_r = wcur["wq"], wcur["wk"], wcur["wv"], wcur["wo"]
        w1r, w2r = mcur

        y = lp.tile([P, DT, W], MDT, tag="y")
        # LN1 on own tokens (halo part deferred until the AllGather landed)
        emit_ln(srcs=[(lambda dt: x[:, dt, :], HALF, CHUNK)], y=y)

        # --- Q projection (own tokens only)
        qr = lp.tile([P, DT, CHUNK], MDT, tag="qr")
        for do in range(DT):
            pq = ps_a.tile([P, CHUNK], F32, tag="ps_a")
            for dt in range(DT):
                nc.tensor.matmul(out=pq[:], lhsT=wq_r[:, dt, ts(do, P)],
                                 rhs=y[:, dt, ds(HALF, CHUNK)],
                                 start=(dt == 0), stop=(dt == DT - 1))
            nc.scalar.activation(qr[:, do, :], pq[:], AF.Identity,
                                 bias=bqt[:, l, do:do + 1], scale=1.0)

        # --- K/V projections, own tokens
        kr = lp.tile([P, DT, W], MDT, tag="kr")
        for do in range(DT):
            pk = ps_a.tile([P, CHUNK], F32, tag="ps_a")
            for dt in range(DT):
                nc.tensor.matmul(out=pk[:], lhsT=wk_r[:, dt, ts(do, P)],
                                 rhs=y[:, dt, ds(HALF, CHUNK)],
                                 start=(dt == 0), stop=(dt == DT - 1))
            nc.scalar.activation(kr[:, do, ds(HALF, CHUNK)], pk[:], AF.Identity,
                                 bias=bkt[:, l, do:do + 1], scale=1.0)

        vt = [lp.tile([P, H * (DH + 1)], MDT, tag=f"vt{t}", name=f"vt{t}") for t in range(W // P)]

        def emit_v(t, ysrc):
            pv = ps_a.tile([P, D], F32, tag="ps_a")
            for dt in range(DT):
                nc.tensor.matmul(out=pv[:], lhsT=ysrc(dt, t), rhs=wv_r[:, dt, :],
                                 start=(dt == 0), stop=(dt == DT - 1))
            vtv = vt[t][:].rearrange("p (h c) -> p h c", c=DH + 1)
            nc.vector.tensor_add(
                out=vtv[:, :, 0:DH],
                in0=pv[:].rearrange("p (h c) -> p h c", c=DH),
                in1=bvt[:, l, :].rearrange("p (h c) -> p h c", c=DH))
            nc.vector.tensor_copy(out=vtv[:, :, DH:DH + 1], in_=ones[:, 0:H])

        for t in range(2, W // P):
            emit_v(t, lambda dt, t: y[:, dt, ts(t, P)])

        # --- sliding-window attention
        attr = lp.tile([P, DT, CHUNK], MDT, tag="attr")
        ej_keep = {}

        def emit_ej(h, kt, keep):
            _, q0, w = KTW[kt]
            r0, dto = (h % 2) * DH, h // 2
            pscore = ps_b.tile([P, 512], F32, tag="ps_b")
            nc.tensor.matmul(out=pscore[:, 0:w],
                             lhsT=kr[ds(r0, DH), dto, ts(kt, P)],
                             rhs=qr[ds(r0, DH), dto, ds(q0, w)],
                             start=True, stop=True)
            ej = lp3.tile([P, 512], MDT, tag="ej_keep" if keep else "ej_tmp",
                          bufs=2 * H if keep else 4, name="ej")
            nc.scalar.activation(ej[:, 0:w], pscore[:, 0:w], AF.Exp,
                                 bias=negb[:], scale=SCALE)
            nc.vector.tensor_mul(out=ej[:, 0:w], in0=ej[:, 0:w], in1=maskt[:, kt, 0:w])
            return ej

        def emit_qblock(h, qB, ejs):
            r0, dto = (h % 2) * DH, h // 2
            pa = ps_c.tile([DH + 1, 256], F32, tag="ps_c")
            for i, kt in enumerate(range(qB * 2, qB * 2 + 4)):
                c0 = qB * 256 - KTW[kt][1]
                nc.tensor.matmul(out=pa[:], lhsT=vt[kt][:, ds(h * (DH + 1), DH + 1)],
                                 rhs=ejs[kt][:, ds(c0, 256)],
                                 start=(i == 0), stop=(i == 3))
            # reciprocal_approx_* misreads PSUM at base_partition!=0 — stage
            # the denominator row to a partition-0 SBUF tile first.
            srow = lp3.tile([1, 256], F32, tag="srow", bufs=2)
            nc.scalar.copy(out=srow[:], in_=pa[DH:DH + 1, :])
            rr = lp3.tile([1, 256], F32, tag="rr", bufs=2)
            nc.vector.reciprocal_approx_fast(out=rr[:], in_=srow[:])
            rr16 = lp3.tile([1, 256], MDT, tag="rr16", bufs=2)
            nc.scalar.copy(out=rr16[:], in_=rr[:])
            pbc = ps_a.tile([DH, 256], F32, tag="ps_a")
            nc.tensor.matmul(out=pbc[:], lhsT=ones[0:1, 0:DH], rhs=rr16[:],
                             start=True, stop=True)
            dst = attr[ds(r0, DH), dto, ds(qB * 256, 256)]
            bcs = lp3.tile([DH, 256], MDT, tag="bcs", bufs=3)
            nc.scalar.copy(out=bcs[:], in_=pbc[:])
            nc.vector.tensor_mul(out=dst, in0=pa[0:DH, :], in1=bcs[:])

        # --- helpers for token-half processing (c0 = 0 or 256)
        def emit_oproj_half(c0):
            for do in range(DT):
                po = ps_a.tile([P, HALF], F32, tag="ps_a")
                for dt in range(DT):
                    nc.tensor.matmul(out=po[:], lhsT=wo_r[:, dt, ts(do, P)],
                                     rhs=attr[:, dt, ds(c0, HALF)],
                                     start=(dt == 0), stop=(dt == DT - 1))
                nc.vector.tensor_add(out=x[:, do, ds(c0, HALF)],
                                     in0=x[:, do, ds(c0, HALF)], in1=po[:])

        def emit_mlp_half(c0):
            emit_ln(srcs=[(lambda dt: x[:, dt, ds(c0, HALF)], c0, HALF)], y=y2)
            pb = [ps_b.tile([P, HALF], F32, tag="ps_b", name=f"pb{i}") for i in range(DT)]

            def emit_mlp2(m, hm):
                for do in range(DT):
                    nc.tensor.matmul(out=pb[do][:], lhsT=w2r[:, m, ts(do, P)],
                                     rhs=hm[:], start=(m == 0), stop=(m == MT - 1))

            hist = []
            for m in range(MT):
                p1 = ps_a.tile([P, HALF], F32, tag="ps_a")
                for dt in range(DT):
                    nc.tensor.matmul(out=p1[:], lhsT=w1r[:, dt, ts(m, P)],
                                     rhs=y2[:, dt, ds(c0, HALF)],
                                     start=(dt == 0), stop=(dt == DT - 1))
                hm = lp3.tile([P, HALF], MDT, tag="hm", bufs=4)
                nc.scalar.activation(hm[:], p1[:], AF.Gelu_apprx_tanh,
                                     bias=b1t[:, l, m:m + 1], scale=1.0)
                hist.append((m, hm))
                if len(hist) > 2:
                    emit_mlp2(*hist.pop(0))
            for mm_, hh_ in hist:
                emit_mlp2(mm_, hh_)
            for do in range(DT):
                nc.vector.scalar_tensor_tensor(
                    out=x[:, do, ds(c0, HALF)], in0=pb[do][:],
                    scalar=b2t[:, l, do:do + 1], in1=x[:, do, ds(c0, HALF)],
                    op0=OP.add, op1=OP.add)

        y2 = lp.tile([P, DT, CHUNK], MDT, tag="y2")

        # import the halo gathered during the previous layer (placed before
        # this layer's collective in the gpsimd queue; waits only on its data)
        if prev_agout is not None:
            for dt in range(DT):
                nc.gpsimd.indirect_dma_start(
                    out=xh[:, dt, :], out_offset=None, in_=prev_agout[:],
                    in_offset=bass.IndirectOffsetOnAxis(ap=hoffs[:, dt:dt + 1], axis=0))

        # phase 1: own-key work for all heads (kt 2..5, qB=1)
        for h in range(H):
            ejs = {kt: emit_ej(h, kt, kt in (2, 3)) for kt in (2, 3, 4, 5)}
            ej_keep[h] = {kt: ejs[kt] for kt in (2, 3)}
            emit_qblock(h, 1, ejs)

        # second token-half completes through its residual, then exports
        emit_oproj_half(HALF)
        emit_mlp_half(HALF)
        if l < NL - 1:
            agin = drp.tile([D, HALF], F32R, tag=f"agin{l}")
            agout = drp.tile([len(GROUPS[0]) * D, HALF], F32R, tag=f"agout{l}")
            nc.sync.dma_start(out=agin[:].rearrange("(t p) m -> p t m", p=P),
                              in_=x[:, :, ds(HALF, HALF)])
            nc.gpsimd.collective_compute(
                "AllGather", OP.bypass, replica_groups=GROUPS,
                ins=[agin.opt()], outs=[agout.opt()])
            prev_agout = agout

        # halo: LN1 on xh, K/V halo columns (consumes prev layer's AllGather)
        emit_ln(srcs=[(lambda dt: xh[:, dt, :], 0, HALF)], y=y)
        for do in range(DT):
            pk = ps_a.tile([P, HALF], F32, tag="ps_a")
            for dt in range(DT):
                nc.tensor.matmul(out=pk[:], lhsT=wk_r[:, dt, ts(do, P)],
                                 rhs=y[:, dt, ds(0, HALF)],
                                 start=(dt == 0), stop=(dt == DT - 1))
            nc.scalar.activation(kr[:, do, ds(0, HALF)], pk[:], AF.Identity,
                                 bias=bkt[:, l, do:do + 1], scale=1.0)
        for t in range(2):
            emit_v(t, lambda dt, t: y[:, dt, ts(t, P)])

        # phase 2: halo-key work (kt 0,1 + kept kt 2,3; qB=0)
        for h in range(H):
            ejs = dict(ej_keep[h])
            for kt in (0, 1):
                ejs[kt] = emit_ej(h, kt, False)
            emit_qblock(h, 0, ejs)

        if l == 0 and "ydump" in io:
            for nm_t, src_t in (("ydump", y), ("qdump", qr), ("kdump", kr), ("adump", attr)):
                nc.sync.dma_start(out=io[nm_t].ap().rearrange("(t p) m -> p t m", p=P),
                                  in_=src_t[:])

        # prefetch next layer's attention weights (double-buffered pool)
        if l + 1 < _knl:
            wcur = load_qkvo(l + 1)

        # first token-half completes
        emit_oproj_half(0)
        emit_mlp_half(0)

        # prefetch next layer's MLP weights (single buffer: reallocates after use)
        if l + 1 < _knl:
            mcur = load_mlp(l + 1)
        dump_x(l + 1)

    # ------------------------------------------------ final LN + allgather
    emit_ln(srcs=[(lambda dt: x[:, dt, :], 0, CHUNK)], y=yf)
    yfd = drp.tile([D, CHUNK], MDT, tag="yfd")
    nc.sync.dma_start(out=yfd[:].rearrange("(t p) m -> p t m", p=P), in_=yf[:])
    yfg = drp.tile([NCORES * D, CHUNK], MDT, tag="yfg", addr_space="Shared")
    nc.gpsimd.collective_compute(
        "AllGather", OP.bypass, replica_groups=[list(range(NCORES))],
        ins=[yfd.opt()], outs=[yfg.opt()])

    # w_out cache: issued here, but the sync queue reaches it during the last
    # layer's compute, so the 4MB load overlaps.
    wof = wofp.tile([P, DT, VSH], MDT, tag="wof")
    nc.sync.dma_start(out=wof[:], in_=io["w_out_sl"].ap().rearrange("(t p) m -> p t m", p=P))

    lp3.release()
    lp.release()

    # ------------------------------------------------ vocab-sharded logits
    # slot 0 = own tokens (local yf, overlaps the AllGather); slot j>0 = core
    # (c+j)%8's tokens, imported via per-core indirect offsets.
    with tc.tile_pool(name="final", bufs=1) as fpool, \
         tc.tile_pool(name="ftrans", bufs=4) as ftp:
        yall = fpool.tile([P, 7, DT, CHUNK], MDT, tag="yall")

        def emit_slot(j, rhs_of):
            for v_i in range(VSH // P):
                pf = ps_a.tile([P, CHUNK], F32, tag="ps_a")
                for dt in range(DT):
                    nc.tensor.matmul(out=pf[:], lhsT=wof[:, dt, ts(v_i, P)],
                                     rhs=rhs_of(dt), start=(dt == 0), stop=(dt == DT - 1))
                ot = ftp.tile([P, CHUNK], F16, tag="fot")
                if v_i % 2 == 0:
                    nc.scalar.activation(ot[:], pf[:], AF.Identity,
                                         bias=bot[:, v_i:v_i + 1], scale=1.0)
                else:
                    nc.vector.tensor_scalar_add(out=ot[:], in0=pf[:],
                                                scalar1=bot[:, v_i:v_i + 1])
                nc.sync.dma_start(out=io["out"].ap()[ts(v_i, P), ts(j, CHUNK)],
                                  in_=ot[:])

        emit_slot(0, lambda dt: yf[:, dt, :])
        for j in range(1, NCORES):
            for dt in range(DT):
                nc.gpsimd.indirect_dma_start(
                    out=yall[:, j - 1, dt, :], out_offset=None, in_=yfg[:],
                    in_offset=bass.IndirectOffsetOnAxis(
                        ap=yoffs[:, (j - 1) * DT + dt:(j - 1) * DT + dt + 1], axis=0))
            emit_slot(j, lambda dt, j=j: yall[:, j - 1, dt, :])

    wofp.release()
    drp.release()
    ps_c.release()
    ps_b.release()
    ps_a.release()
    wmlp.release()
    wqk.release()
    xpool.release()
    cpool.release()


# ================================================================ host side
def _pe_table():
    pos = np.arange(S, dtype=np.float32)[:, None]
    div = np.exp(np.arange(0, D, 2, dtype=np.float32) * -(np.log(10000.0) / D))
    pe = np.zeros((S, D), dtype=np.float32)
    pe[:, 0::2] = np.sin(pos * div)
    pe[:, 1::2] = np.cos(pos * div)
    return pe


def _in_maps(inputs):
    inp = np.asarray(inputs["inputs"]).astype(np.int32)
    ids = np.pad(inp, ((0, 0), (1, 0)))[:, :-1].astype(np.int32)
    pe = _pe_table()

    f32 = lambda k: np.asarray(inputs[k], dtype=np.float32)
    ln1_s, ln1_b = f32("ln1_s"), f32("ln1_b")
    ln2_s, ln2_b = f32("ln2_s"), f32("ln2_b")
    lnf_s, lnf_b = f32("lnf_s").reshape(D), f32("lnf_b").reshape(D)
    wq, wk, wv, wo = f32("wq"), f32("wk"), f32("wv"), f32("wo")
    w1, w2 = f32("w1"), f32("w2")
    b1, b2 = f32("b1"), f32("b2")
    wout, bout = f32("w_out"), f32("b_out")

    # fold LN affine into the downstream projections
    wq_f = wq * ln1_s[:, :, None]
    wk_f = wk * ln1_s[:, :, None]
    wv_f = wv * ln1_s[:, :, None]
    w1_f = w1 * ln2_s[:, :, None]
    bq = np.einsum("ld,ldm->lm", ln1_b, wq)
    bk = np.einsum("ld,ldm->lm", ln1_b, wk)
    bv = np.einsum("ld,ldm->lm", ln1_b, wv)
    b1_f = b1 + np.einsum("ld,ldm->lm", ln2_b, w1)
    wout_f = wout * lnf_s[:, None]
    bout_f = bout + lnf_b @ wout

    shared = {
        "embed": np.ascontiguousarray(f32("embed")),
        "b1": b1_f, "b2": b2, "bq": bq, "bk": bk,
        "bv": np.ascontiguousarray(np.broadcast_to(bv[None], (P, NL, D))),
        "wq": wq_f.astype(np.float16), "wk": wk_f.astype(np.float16),
        "wv": wv_f.astype(np.float16), "wo": wo.astype(np.float16),
        "w1": w1_f.astype(np.float16), "w2": w2.astype(np.float16),
    }
    shared = {k: np.ascontiguousarray(v) for k, v in shared.items()}
    wout16 = wout_f.astype(np.float16)

    maps = []
    for c in range(NCORES):
        b, ch = divmod(c, NCORES // B)
        t0 = ch * CHUNK
        lo = t0 - HALF
        ids768 = np.zeros(W, np.int32)
        pe768 = np.zeros((W, D), np.float32)
        s0 = max(0, lo)
        ids768[s0 - lo:] = ids[b, s0:t0 + CHUNK]
        pe768[s0 - lo:] = pe[s0:t0 + CHUNK]
        # per-key-tile masks: [6, 128, 512] f16
        m = np.zeros((6, P, 512), np.float16)
        for kt, q0, w in KTW:
            uk = kt * P + np.arange(P)[:, None]
            q = q0 + np.arange(w)[None, :]
            dqk = (HALF + q) - uk
            ok = (dqk >= 0) & (dqk <= HALF)
            if ch == 0:
                ok = ok & ((lo + uk) >= 0)
            m[kt, :, :w] = ok.astype(np.float16)
        src = ch - 1 if ch > 0 else 0
        hoffs = (src * D + np.arange(DT)[None, :] * P
                 + np.arange(P)[:, None]).astype(np.int32)
        yo = np.zeros((P, 7 * DT), np.int32)
        for j in range(1, NCORES):
            sc = (c + j) % NCORES
            for dt in range(DT):
                yo[:, (j - 1) * DT + dt] = sc * D + dt * P + np.arange(P)
        vlo = c * VV
        wsl = np.zeros((D, VSH), np.float16)
        wsl[:, :VV] = wout16[:, vlo:vlo + VV]
        bsl = np.zeros((1, VSH), np.float32)
        bsl[0, :VV] = bout_f[vlo:vlo + VV]
        mp = dict(shared)
        mp.update(
            idx_in=np.ascontiguousarray(ids768.reshape(W // P, P).T),
            pe_dm=np.ascontiguousarray(pe768.T),
            masks=m, halo_offs=hoffs, yoffs=yo, w_out_sl=wsl, b_out_sl=bsl)
        maps.append(mp)
    return maps


def _assemble(res):
    full = np.empty((NTOK, V), np.float32)
    for c in range(NCORES):
        lv = np.asarray(res[c]["logits_vm"], dtype=np.float32)  # [VSH, NTOK] rotated
        for j in range(NCORES):
            blk = (c + j) % NCORES
            full[blk * CHUNK:(blk + 1) * CHUNK, c * VV:(c + 1) * VV] = \
                lv[:VV, j * CHUNK:(j + 1) * CHUNK].T
    return full.reshape(B, S, V)


def kernel(**inputs):
    nc = _CACHE.get("nc")
    if nc is None:
        nc = _build()
        _CACHE["nc"] = nc
    maps = _in_maps(inputs)
    res = run_bass_kernel_spmd(nc, maps, list(range(NCORES))).results
    return _assemble(res)
